# revision 4
# baseline (speedup 1.0000x reference)
"""Context-Query Attention (BiDAF-style trilinear attention) on 8 Trainium2 cores.

Data-parallel over batch: 64 examples -> 8 per NeuronCore. Per example
(cont=400, ques=50, d=128):
  S[i,j] = <c_i, w_c> + <q_j, w_q> + <c_i * w_cq, q_j>          [400, 50]
  S_q = masked softmax over j, S_c = masked softmax over i (both * mask)
  a = S_q @ q ; b = S_q @ (S_c^T @ c)
  x = [c, a, c*a, c*b]                                          [400, 512]

Device-side layout tricks:
  - masks folded into exp() as additive -1e30 biases (no separate mask ops)
  - softmax-over-j denominator from activation accum_out (free)
  - softmax-over-i denominator from a ones-column appended to c (free)
  - w_cq folded into q^T (qwT) so S^T is a single K=128 matmul + K=1 bias mm
"""

import os
from contextlib import ExitStack

import numpy as np

import concourse.bacc as bacc
import concourse.bass as bass
import concourse.tile as tile
from concourse import mybir
from concourse.bass_utils import run_bass_kernel_spmd
from concourse.masks import make_identity

F32 = mybir.dt.float32
B, CONT, QUES, D = 64, 400, 50, 128
M = 8                   # cores
BE = B // M             # examples per core
NK, PK = 4, 100         # cont chunking: 4 chunks x 100 rows
OC = 4 * D + 2 * QUES   # 612 output columns: [c | a | c*a | c*b | S_q | S_c]
NEG = -1.0e30


def _body(ctx: ExitStack, tc: "tile.TileContext", aps: dict):
    nc = tc.nc
    cT, cn, qT, qn, cml, qml, wp, out = (
        aps["cT"], aps["cn"], aps["qT"], aps["qn"],
        aps["cmlp"], aps["qml"], aps["wpack"], aps["out"],
    )

    const = ctx.enter_context(tc.tile_pool(name="const", bufs=1))
    work = ctx.enter_context(tc.tile_pool(name="work", bufs=2))
    outp = ctx.enter_context(tc.tile_pool(name="outp", bufs=2))
    ps_st = ctx.enter_context(tc.tile_pool(name="ps_st", bufs=1, space="PSUM"))
    ps_tr = ctx.enter_context(tc.tile_pool(name="ps_tr", bufs=2, space="PSUM"))
    ps_ab = ctx.enter_context(tc.tile_pool(name="ps_ab", bufs=2, space="PSUM"))
    ps_u = ctx.enter_context(tc.tile_pool(name="ps_u", bufs=1, space="PSUM"))
    ps_sm = ctx.enter_context(tc.tile_pool(name="ps_sm", bufs=2, space="PSUM"))

    # ---- static loads (whole-core inputs, one big DMA each) ----
    s_cT = const.tile([D, BE, CONT], F32)
    nc.sync.dma_start(out=s_cT[:], in_=cT)
    s_cn = const.tile([PK, BE, NK, D + 1], F32)
    nc.sync.dma_start(out=s_cn[:], in_=cn)
    s_qT = const.tile([D, BE, QUES], F32)
    nc.sync.dma_start(out=s_qT[:], in_=qT)
    s_qn = const.tile([QUES, BE, D], F32)
    nc.sync.dma_start(out=s_qn[:], in_=qn)
    s_cml = const.tile([PK, BE, NK], F32)
    nc.sync.dma_start(out=s_cml[:], in_=cml)
    s_qml = const.tile([1, BE, QUES], F32)
    nc.sync.dma_start(out=s_qml[:], in_=qml)
    s_w = const.tile([D, 3], F32)
    nc.sync.dma_start(out=s_w[:], in_=wp)

    ident = const.tile([D, D], F32)
    make_identity(nc, ident[:])
    ones_row = const.tile([1, CONT], F32)
    nc.vector.memset(ones_row[:], 1.0)

    EXP = mybir.ActivationFunctionType.Exp

    for e in range(BE):
        qTe = s_qT[:, e, :]                       # [128, 50] d-major
        # qwT = qT * w_cq  (fold w_cq into the stationary operand)
        qwT = work.tile([D, QUES], F32, tag="qwT")
        nc.vector.tensor_scalar_mul(qwT[:], qTe, s_w[:, 2:3])

        # bias_q row [1, 50] = w_q^T @ qT  (+ q-mask log)
        bq_ps = ps_sm.tile([1, QUES], F32, tag="sm")
        nc.tensor.matmul(bq_ps[:], s_w[:, 1:2], qTe, start=True, stop=True)
        bqm = work.tile([1, QUES], F32, tag="bqm")
        nc.vector.tensor_add(bqm[:], bq_ps[:], s_qml[:, e, :])

        # bias_c partition-layout [100, 4] = cT^T @ w_c per chunk (+ c-mask log)
        bc_ps = ps_sm.tile([PK, NK], F32, tag="sm")
        for k in range(NK):
            nc.tensor.matmul(
                bc_ps[:, k : k + 1],
                s_cT[:, e, PK * k : PK * (k + 1)],
                s_w[:, 0:1],
                start=True, stop=True,
            )
        bcm = work.tile([PK, NK], F32, tag="bcm")
        nc.vector.tensor_add(bcm[:], bc_ps[:], s_cml[:, e, :])

        # S^T [50, 400] = qwT^T @ cT  + bias_q broadcast along i
        st_ps = ps_st.tile([QUES, CONT], F32, tag="st")
        nc.tensor.matmul(st_ps[:], qwT[:], s_cT[:, e, :], start=True, stop=False)
        nc.tensor.matmul(st_ps[:], bqm[:], ones_row[:], start=False, stop=True)
        st_sb = work.tile([QUES, CONT], F32, tag="st_sb")
        nc.scalar.copy(st_sb[:], st_ps[:])

        # transpose to natural chunks, exp with bias_c(+mask); accum -> Dq
        sn_ps = ps_tr.tile([PK, NK * QUES], F32, tag="tr")
        En = work.tile([PK, NK, QUES], F32, tag="En")
        Dq = work.tile([PK, NK], F32, tag="Dq")
        for k in range(NK):
            nc.tensor.transpose(
                sn_ps[:, k * QUES : (k + 1) * QUES],
                st_sb[:, PK * k : PK * (k + 1)],
                ident[0:QUES, 0:QUES],
            )
            nc.scalar.activation(
                En[:, k, :],
                sn_ps[:, k * QUES : (k + 1) * QUES],
                EXP,
                bias=bcm[:, k : k + 1],
                scale=1.0,
                accum_out=Dq[:, k : k + 1],
            )

        Rq = work.tile([PK, NK], F32, tag="Rq")
        nc.vector.tensor_scalar_add(Rq[:], Dq[:], 1e-30)
        nc.vector.reciprocal(Rq[:], Rq[:])

        ot = outp.tile([PK, NK, OC], F32, tag="ot")

        # S_q = En * Rq (per-row); transpose back for the a/b matmuls
        sqT_ps = ps_tr.tile([QUES, CONT], F32, tag="tr")
        sqT = work.tile([QUES, CONT], F32, tag="sqT")
        for k in range(NK):
            nc.vector.tensor_scalar_mul(
                ot[:, k, 4 * D : 4 * D + QUES], En[:, k, :], Rq[:, k : k + 1]
            )
            nc.tensor.transpose(
                sqT_ps[:, PK * k : PK * (k + 1)],
                ot[:, k, 4 * D : 4 * D + QUES],
                ident[0:PK, 0:PK],
            )
            nc.scalar.copy(
                sqT[:, PK * k : PK * (k + 1)], sqT_ps[:, PK * k : PK * (k + 1)]
            )

        # u_raw [50, 129] = E^T @ [c | 1]; col 128 = Dc (softmax-over-i denom)
        u_ps = ps_u.tile([QUES, D + 1], F32, tag="u")
        for k in range(NK):
            nc.tensor.matmul(
                u_ps[:], En[:, k, :], s_cn[:, e, k, :],
                start=(k == 0), stop=(k == NK - 1),
            )
        Rc = work.tile([QUES, 1], F32, tag="Rc")
        nc.vector.tensor_scalar_add(Rc[:], u_ps[:, D : D + 1], 1e-30)
        nc.vector.reciprocal(Rc[:], Rc[:])
        u_sb = work.tile([QUES, D], F32, tag="u_sb")
        nc.scalar.mul(u_sb[:], u_ps[:, 0:D], Rc[:])

        # S_c = En * Rc (per-col): broadcast Rc across partitions first
        rc_ps = ps_sm.tile([1, QUES], F32, tag="sm")
        nc.tensor.transpose(rc_ps[:], Rc[:], ident[0:QUES, 0:QUES])
        rc_row = work.tile([1, QUES], F32, tag="rcrow")
        nc.scalar.copy(rc_row[:], rc_ps[:])
        rc_b = work.tile([PK, QUES], F32, tag="rcb")
        nc.gpsimd.partition_broadcast(rc_b[:], rc_row[:])
        for k in range(NK):
            nc.vector.tensor_mul(
                ot[:, k, 4 * D + QUES :], En[:, k, :], rc_b[:]
            )

        # a = S_q @ q ; b = S_q @ u ; x = [c, a, c*a, c*b]
        for k in range(NK):
            a_ps = ps_ab.tile([PK, D], F32, tag="ab")
            nc.tensor.matmul(
                a_ps[:], sqT[:, PK * k : PK * (k + 1)], s_qn[:, e, :],
                start=True, stop=True,
            )
            nc.scalar.copy(ot[:, k, D : 2 * D], a_ps[:])
            nc.vector.tensor_mul(
                ot[:, k, 2 * D : 3 * D], s_cn[:, e, k, 0:D], a_ps[:]
            )
            b_ps = ps_ab.tile([PK, D], F32, tag="ab")
            nc.tensor.matmul(
                b_ps[:], sqT[:, PK * k : PK * (k + 1)], u_sb[:],
                start=True, stop=True,
            )
            nc.vector.tensor_mul(
                ot[:, k, 3 * D : 4 * D], s_cn[:, e, k, 0:D], b_ps[:]
            )
            nc.gpsimd.tensor_copy(ot[:, k, 0:D], s_cn[:, e, k, 0:D])

        nc.sync.dma_start(
            out=out[e].rearrange("k p c -> p k c"), in_=ot[:]
        )


_CACHE = {}


def _build():
    if "nc" in _CACHE:
        return _CACHE["nc"]
    nc = bacc.Bacc(
        "TRN2", target_bir_lowering=False, debug=False,
        enable_asserts=False, num_devices=M,
    )
    aps = {
        "cT": nc.dram_tensor("cT", [D, BE, CONT], F32, kind="ExternalInput").ap(),
        "cn": nc.dram_tensor("cn", [PK, BE, NK, D + 1], F32, kind="ExternalInput").ap(),
        "qT": nc.dram_tensor("qT", [D, BE, QUES], F32, kind="ExternalInput").ap(),
        "qn": nc.dram_tensor("qn", [QUES, BE, D], F32, kind="ExternalInput").ap(),
        "cmlp": nc.dram_tensor("cmlp", [PK, BE, NK], F32, kind="ExternalInput").ap(),
        "qml": nc.dram_tensor("qml", [1, BE, QUES], F32, kind="ExternalInput").ap(),
        "wpack": nc.dram_tensor("wpack", [D, 3], F32, kind="ExternalInput").ap(),
        "out": nc.dram_tensor("out", [BE, NK, PK, OC], F32, kind="ExternalOutput").ap(),
    }
    with tile.TileContext(nc) as tc, ExitStack() as ctx:
        _body(ctx, tc, aps)
    nc.compile()
    _CACHE["nc"] = nc
    return nc


def _pack_core(c8, q8, cl8, ql8, W):
    f32 = np.float32
    cT_h = np.ascontiguousarray(c8.transpose(2, 0, 1), dtype=f32)
    cnat = c8.reshape(BE, NK, PK, D).transpose(2, 0, 1, 3)
    cn_h = np.empty([PK, BE, NK, D + 1], dtype=f32)
    cn_h[..., :D] = cnat
    cn_h[..., D] = 1.0
    qT_h = np.ascontiguousarray(q8.transpose(2, 0, 1), dtype=f32)
    qn_h = np.ascontiguousarray(q8.transpose(1, 0, 2), dtype=f32)
    cmlf = np.where(np.arange(CONT)[None, :] < cl8, 0.0, NEG).astype(f32)
    cml_h = np.ascontiguousarray(cmlf.reshape(BE, NK, PK).transpose(2, 0, 1))
    qml_h = np.ascontiguousarray(
        np.where(np.arange(QUES)[None, :] < ql8, 0.0, NEG).astype(f32)[None]
    )
    wp_h = np.ascontiguousarray(W.reshape(3, D).T, dtype=f32)
    return {
        "cT": cT_h, "cn": cn_h, "qT": qT_h, "qn": qn_h,
        "cmlp": cml_h, "qml": qml_h, "wpack": wp_h,
    }


def _make_in_maps(c, q, c_len, q_len, W):
    c = np.asarray(c, dtype=np.float32)
    q = np.asarray(q, dtype=np.float32)
    c_len = np.asarray(c_len)
    q_len = np.asarray(q_len)
    W = np.asarray(W, dtype=np.float32)
    return [
        _pack_core(
            c[m * BE : (m + 1) * BE],
            q[m * BE : (m + 1) * BE],
            c_len[m * BE : (m + 1) * BE],
            q_len[m * BE : (m + 1) * BE],
            W,
        )
        for m in range(M)
    ]


def _unpack(results):
    x = np.empty([B, CONT, 4 * D], dtype=np.float32)
    S_q = np.empty([B, CONT, QUES], dtype=np.float32)
    S_c = np.empty([B, CONT, QUES], dtype=np.float32)
    for m, res in enumerate(results):
        O = res["out"].reshape(BE, CONT, OC)
        x[m * BE : (m + 1) * BE] = O[..., : 4 * D]
        S_q[m * BE : (m + 1) * BE] = O[..., 4 * D : 4 * D + QUES]
        S_c[m * BE : (m + 1) * BE] = O[..., 4 * D + QUES :]
    return x, S_q, S_c


def run_raw(c, q, c_len, q_len, W, **run_kwargs):
    nc = _build()
    in_maps = _make_in_maps(c, q, c_len, q_len, W)
    return run_bass_kernel_spmd(nc, in_maps, list(range(M)), **run_kwargs)


def kernel(c, q, c_len, q_len, W):
    return _unpack(run_raw(c, q, c_len, q_len, W).results)


# revision 17
# speedup vs baseline: 1.1195x; 1.1195x over previous
"""Context-Query Attention (BiDAF-style trilinear attention) on 8 Trainium2 cores.

Data-parallel over batch: 64 examples -> 8 per NeuronCore. Per example
(cont=400, ques=50, d=128):
  S[i,j] = <c_i, w_c> + <q_j, w_q> + <c_i * w_cq, q_j>          [400, 50]
  S_q = masked softmax over j, S_c = masked softmax over i (both * mask)
  a = S_q @ q ; b = S_q @ (S_c^T @ c)
  x = [c, a, c*a, c*b]                                          [400, 512]

Factorized device dataflow (masks/biases folded, minimal PE instructions):
  E'[j,i] = exp(S_core[j,i] + bq[j] + qmask_log[j])   one [50,400] matmul + exp
  ec[i]   = exp(bc[i] + cmask_log[i])                 (biases via DVE dot-reduce)
  E_nat   = transpose(E') * ec                        (PE transpose + ACT scale copy,
                                                       accum_out -> Dq for free)
  u_raw   = E_nat^T @ [c|1] -> col 128 = Dc           (Dc for free)
  a|b     = E'-chunk @ [q|u] one matmul, post-scaled by rr = ec/Dq
"""

import os
from contextlib import ExitStack

import numpy as np

import concourse.bacc as bacc
import concourse.bass as bass
import concourse.tile as tile
from concourse import mybir
from concourse.bass_utils import run_bass_kernel_spmd
from concourse.masks import make_identity

F32 = mybir.dt.float32
B, CONT, QUES, D = 64, 400, 50, 128
M = 8                   # cores
BE = B // M             # examples per core
NK, PK = 4, 100         # cont chunking: 4 chunks x 100 rows
OC = 4 * D + 2 * QUES   # 612 output columns: [c | a | c*a | c*b | S_q | S_c]
NEG = -1.0e30
TINY = 1.0e-30


def _body(ctx: ExitStack, tc: "tile.TileContext", aps: dict):
    nc = tc.nc
    cT, cn, qT, qn, cm, qml, wp, wrow, out = (
        aps["cT"], aps["cn"], aps["qT"], aps["qn"],
        aps["cmp"], aps["qmlc"], aps["wpack"], aps["wrow"], aps["out"],
    )
    EXP = mybir.ActivationFunctionType.Exp
    MUL = mybir.AluOpType.mult

    const = ctx.enter_context(tc.tile_pool(name="const", bufs=1))
    work = ctx.enter_context(tc.tile_pool(name="work", bufs=2))
    outp = ctx.enter_context(tc.tile_pool(name="outp", bufs=2))
    ps_st = ctx.enter_context(tc.tile_pool(name="ps_st", bufs=1, space="PSUM"))
    ps_tr = ctx.enter_context(tc.tile_pool(name="ps_tr", bufs=1, space="PSUM"))
    ps_ab = ctx.enter_context(tc.tile_pool(name="ps_ab", bufs=2, space="PSUM"))
    ps_u = ctx.enter_context(tc.tile_pool(name="ps_u", bufs=1, space="PSUM"))
    ps_sm = ctx.enter_context(tc.tile_pool(name="ps_sm", bufs=1, space="PSUM"))

    # ---- static loads (whole-core inputs, one big DMA each) ----
    s_cT = const.tile([D, BE, CONT], F32)
    nc.sync.dma_start(out=s_cT[:], in_=cT)
    s_cn = const.tile([PK, BE, NK, D + 1], F32)
    nc.sync.dma_start(out=s_cn[:], in_=cn)
    s_qT = const.tile([D, BE, QUES], F32)
    nc.sync.dma_start(out=s_qT[:], in_=qT)
    s_qn = const.tile([QUES, BE, D], F32)
    nc.sync.dma_start(out=s_qn[:], in_=qn)
    s_cm = const.tile([PK, BE, NK], F32)
    nc.sync.dma_start(out=s_cm[:], in_=cm)
    s_qml = const.tile([QUES, BE], F32)
    nc.sync.dma_start(out=s_qml[:], in_=qml)
    s_w = const.tile([D, 3], F32)
    nc.sync.dma_start(out=s_w[:], in_=wp)

    ident = const.tile([D, D], F32)
    make_identity(nc, ident[:])
    # broadcast the w_q row across partitions via partition-stride-0 DMA
    wq_b = const.tile([QUES, D], F32)
    nc.gpsimd.dma_start(
        out=wq_b[:], in_=bass.AP(wrow.tensor, wrow.offset, [[0, QUES], [1, D]])
    )

    MQ = QUES + 1  # stationary cols: 50 q's + w_c (bias_c row rides along)

    for e in range(BE):
        # ---- bias_q = q @ w_q on DVE (mul + reduce), fold q-mask log ----
        scr_q = work.tile([QUES, D], F32, tag="scr_q")
        nc.vector.tensor_mul(scr_q[:], s_qn[:, e, :], wq_b[:])
        bqm = work.tile([MQ, 1], F32, tag="bqm")
        nc.vector.memset(bqm[:], 0.0)
        nc.vector.reduce_sum(bqm[0:QUES], scr_q[:], axis=mybir.AxisListType.X)
        nc.vector.tensor_add(bqm[0:QUES], bqm[0:QUES], s_qml[:, e : e + 1])

        # ---- S^T core matmul (+bias_c row), exp with q-side bias folded ----
        qw2 = work.tile([D, MQ], F32, tag="qw2")
        nc.vector.tensor_scalar_mul(qw2[:, 0:QUES], s_qT[:, e, :], s_w[:, 2:3])
        nc.vector.tensor_copy(qw2[:, QUES:MQ], s_w[:, 0:1])
        st_ps = ps_st.tile([MQ, CONT], F32, tag="st")
        nc.tensor.matmul(st_ps[:], qw2[:], s_cT[:, e, :], start=True, stop=True)
        Et = work.tile([MQ, CONT], F32, tag="Et")
        nc.scalar.activation(Et[:], st_ps[:], EXP, bias=bqm[:])

        # ---- natural-layout E chunks: transpose, scale by ec*cmask -> Dq ----
        sn_ps = ps_tr.tile([PK, NK, MQ], F32, tag="tr")
        En = work.tile([PK, NK, QUES], F32, tag="En")
        ecm = work.tile([PK, NK], F32, tag="ecm")
        Dq = work.tile([PK, NK], F32, tag="Dq")
        for k in range(NK):
            nc.tensor.transpose(
                sn_ps[:, k, :],
                Et[:, PK * k : PK * (k + 1)],
                ident[0:MQ, 0:MQ],
            )
            nc.vector.tensor_mul(
                ecm[:, k : k + 1], sn_ps[:, k, QUES:MQ], s_cm[:, e, k : k + 1]
            )
            nc.scalar.activation(
                En[:, k, :], sn_ps[:, k, 0:QUES],
                mybir.ActivationFunctionType.Copy,
                scale=ecm[:, k : k + 1], accum_out=Dq[:, k : k + 1],
            )

        Rq = work.tile([PK, NK], F32, tag="Rq")
        nc.vector.tensor_scalar_add(Rq[:], Dq[:], TINY)
        nc.vector.reciprocal(Rq[:], Rq[:])
        rr = work.tile([PK, NK], F32, tag="rr")
        nc.vector.tensor_mul(rr[:], Rq[:], ecm[:])

        # ---- u_raw = E_nat^T @ [c|1] (col 128 = Dc), scale by Rc ----
        u_ps = ps_u.tile([QUES, D + 1], F32, tag="u")
        for k in range(NK):
            nc.tensor.matmul(
                u_ps[:], En[:, k, :], s_cn[:, e, k, :],
                start=(k == 0), stop=(k == NK - 1),
            )
        Rc = work.tile([QUES, 1], F32, tag="Rc")
        nc.vector.tensor_scalar_add(Rc[:], u_ps[:, D : D + 1], TINY)
        nc.vector.reciprocal(Rc[:], Rc[:])
        qu = work.tile([QUES, 2 * D], F32, tag="qu")
        nc.gpsimd.tensor_copy(qu[:, 0:D], s_qn[:, e, :])
        nc.scalar.mul(qu[:, D : 2 * D], u_ps[:, 0:D], Rc[:])

        # ---- Rc broadcast across partitions for the S_c output ----
        rc_ps = ps_sm.tile([1, QUES], F32, tag="sm")
        nc.tensor.transpose(rc_ps[:], Rc[:], ident[0:QUES, 0:QUES])
        rc_row = work.tile([1, QUES], F32, tag="rcrow")
        nc.scalar.copy(rc_row[:], rc_ps[:])
        rc_b = work.tile([PK, QUES], F32, tag="rcb")
        nc.gpsimd.partition_broadcast(rc_b[:], rc_row[:])

        ot = outp.tile([PK, NK, OC], F32, tag="ot")
        # S_q = En * Rq (ACT per-partition scale); S_c = En * Rc (GP, bcast rows)
        for k in range(NK):
            nc.scalar.mul(
                ot[:, k, 4 * D : 4 * D + QUES], En[:, k, :], Rq[:, k : k + 1]
            )
        nc.vector.tensor_tensor(
            out=ot[:, :, 4 * D + QUES :],
            in0=En[:],
            in1=rc_b[:, None, :].to_broadcast([PK, NK, QUES]),
            op=MUL,
        )

        # ---- a|b = E'-chunk @ [q|u], post-scale by rr ----
        b_sb = work.tile([PK, NK, D], F32, tag="b_sb")
        for k in range(NK):
            ab_ps = ps_ab.tile([PK, 2 * D], F32, tag="ab")
            nc.tensor.matmul(
                ab_ps[:], Et[0:QUES, PK * k : PK * (k + 1)], qu[:],
                start=True, stop=True,
            )
            nc.scalar.mul(ot[:, k, D : 2 * D], ab_ps[:, 0:D], rr[:, k : k + 1])
            nc.scalar.mul(b_sb[:, k, :], ab_ps[:, D : 2 * D], rr[:, k : k + 1])
        # ca, cb, and the c passthrough (merged across chunks)
        nc.vector.tensor_mul(
            ot[:, :, 2 * D : 3 * D], s_cn[:, e, :, 0:D], ot[:, :, D : 2 * D]
        )
        nc.vector.tensor_mul(ot[:, :, 3 * D : 4 * D], s_cn[:, e, :, 0:D], b_sb[:])
        for k in range(NK):
            nc.gpsimd.tensor_copy(out=ot[:, k, 0:D], in_=s_cn[:, e, k, 0:D])

        nc.sync.dma_start(out=out[e].rearrange("k p c -> p k c"), in_=ot[:])


_CACHE = {}


def _build():
    if "nc" in _CACHE:
        return _CACHE["nc"]
    nc = bacc.Bacc(
        "TRN2", target_bir_lowering=False, debug=False,
        enable_asserts=False, num_devices=M,
    )
    aps = {
        "cT": nc.dram_tensor("cT", [D, BE, CONT], F32, kind="ExternalInput").ap(),
        "cn": nc.dram_tensor("cn", [PK, BE, NK, D + 1], F32, kind="ExternalInput").ap(),
        "qT": nc.dram_tensor("qT", [D, BE, QUES], F32, kind="ExternalInput").ap(),
        "qn": nc.dram_tensor("qn", [QUES, BE, D], F32, kind="ExternalInput").ap(),
        "cmp": nc.dram_tensor("cmp", [PK, BE, NK], F32, kind="ExternalInput").ap(),
        "qmlc": nc.dram_tensor("qmlc", [QUES, BE], F32, kind="ExternalInput").ap(),
        "wpack": nc.dram_tensor("wpack", [D, 3], F32, kind="ExternalInput").ap(),
        "wrow": nc.dram_tensor("wrow", [1, D], F32, kind="ExternalInput").ap(),
        "out": nc.dram_tensor("out", [BE, NK, PK, OC], F32, kind="ExternalOutput").ap(),
    }
    with tile.TileContext(nc) as tc, ExitStack() as ctx:
        _body(ctx, tc, aps)
    nc.compile()
    _CACHE["nc"] = nc
    return nc


def _pack_core(c8, q8, cl8, ql8, W):
    f32 = np.float32
    cT_h = np.ascontiguousarray(c8.transpose(2, 0, 1), dtype=f32)
    cnat = c8.reshape(BE, NK, PK, D).transpose(2, 0, 1, 3)
    cn_h = np.empty([PK, BE, NK, D + 1], dtype=f32)
    cn_h[..., :D] = cnat
    cn_h[..., D] = 1.0
    qT_h = np.ascontiguousarray(q8.transpose(2, 0, 1), dtype=f32)
    qn_h = np.ascontiguousarray(q8.transpose(1, 0, 2), dtype=f32)
    cmf = (np.arange(CONT)[None, :] < cl8).astype(f32)
    cm_h = np.ascontiguousarray(cmf.reshape(BE, NK, PK).transpose(2, 0, 1))
    qml_h = np.ascontiguousarray(
        np.where(np.arange(QUES)[None, :] < ql8, 0.0, NEG).astype(f32).T
    )
    wp_h = np.ascontiguousarray(W.reshape(3, D).T, dtype=f32)
    wrow_h = np.ascontiguousarray(W.reshape(3, D)[None, 1, :], dtype=f32)
    return {
        "cT": cT_h, "cn": cn_h, "qT": qT_h, "qn": qn_h,
        "cmp": cm_h, "qmlc": qml_h, "wpack": wp_h, "wrow": wrow_h,
    }


def _make_in_maps(c, q, c_len, q_len, W):
    c = np.asarray(c, dtype=np.float32)
    q = np.asarray(q, dtype=np.float32)
    c_len = np.asarray(c_len)
    q_len = np.asarray(q_len)
    W = np.asarray(W, dtype=np.float32)
    return [
        _pack_core(
            c[m * BE : (m + 1) * BE],
            q[m * BE : (m + 1) * BE],
            c_len[m * BE : (m + 1) * BE],
            q_len[m * BE : (m + 1) * BE],
            W,
        )
        for m in range(M)
    ]


def _unpack(results):
    x = np.empty([B, CONT, 4 * D], dtype=np.float32)
    S_q = np.empty([B, CONT, QUES], dtype=np.float32)
    S_c = np.empty([B, CONT, QUES], dtype=np.float32)
    for m, res in enumerate(results):
        O = res["out"].reshape(BE, CONT, OC)
        x[m * BE : (m + 1) * BE] = O[..., : 4 * D]
        S_q[m * BE : (m + 1) * BE] = O[..., 4 * D : 4 * D + QUES]
        S_c[m * BE : (m + 1) * BE] = O[..., 4 * D + QUES :]
    return x, S_q, S_c


def run_raw(c, q, c_len, q_len, W, **run_kwargs):
    nc = _build()
    in_maps = _make_in_maps(c, q, c_len, q_len, W)
    return run_bass_kernel_spmd(nc, in_maps, list(range(M)), **run_kwargs)


def kernel(c, q, c_len, q_len, W):
    return _unpack(run_raw(c, q, c_len, q_len, W).results)
